# revision 6
# baseline (speedup 1.0000x reference)
"""Multi-scale self-attention (nn_AttentionModule) as a Bass/Tile kernel
on 8 TRN2 NeuronCores.

Problem: for scales (4,2,1): avg-pool x [4,128,64,64] -> [B,C,Hs,Ws],
N=Hs*Ws self-attention with q=k=v=x (C=128 contraction), bilinear
upsample back to 64x64 (half-pixel, edge-clamped), sum over scales.

Sharding: 2 cores per batch element; each core computes half the
queries at every scale (with one overlap row at the coarse scales so
the bilinear upsample is core-local) and produces rows [h*32,(h+1)*32)
of its batch's output. All cores run the identical program; only the
input data differs. For scale 1 the host ROTATES the key order per
core so the query block is always columns [0,2048) of xk1r — no
separate xq1r tensor (attention is permutation-invariant in m).

Per-core algorithm (per scale):
 - scores tile [q=128, m<=1536] = xq^T @ xk  (fp32r matmuls, K=C=128)
 - softmax without a max pass: bias b_q = min(|x_q| * max_m |x_m|,
   |x_q|^2 + 30) >= row max (Cauchy-Schwarz / diagonal), applied as the
   per-partition ACT bias; exp emits unnormalized attn in bf16 with the
   row-sum denominator via accum_out (the small tail chunk sums on DVE
   to spare scalar-engine accumulator reads).
 - attn chunks are DMA-transposed to [m-tile, q] layout; phase B
   accumulates out_unnorm[c,q] = sum_m xkT[m,c]*attnT[m,q].
 - two persistent PSUM banks alternate per group: one accumulates phase
   B while the other stages the 1/denom PE transposes; the partition
   broadcast of 1/denom runs on the (idle) GpSimd engine, so the
   normalizing store costs O(C*N) and never blocks the tensor queue.
 - bilinear upsample + cross-scale sum with strided DVE ops, emitted
   interleaved with scale-1 compute; output DMA split per column range.
"""

import numpy as np
import ml_dtypes

MCHUNK = 1024
S1_GROUPS = [(0, 4), (4, 4), (8, 4), (12, 4)]
SC_BUFS = 2
WARMUP_MM = 10
BCAST_POS = 4          # pair-steps before the bcast unit in phase B

P = 128
B, C, H, W = 4, 128, 64, 64
N1, N2, N4 = 4096, 1024, 256
NQ1 = 2048          # 16 q-tiles (half the image)
NQ2 = 640           # 5 q-tiles = 20 pooled rows (18 needed + 2 pad)
NQ4 = 256           # 2 q-tiles = 16 pooled rows (10 needed + 6 pad)

_BF16 = ml_dtypes.bfloat16


def _build_module():
    import concourse.bacc as bacc
    import concourse.mybir as mybir
    import concourse.tile as tile

    f32 = mybir.dt.float32
    f32r = mybir.dt.float32r
    bf16 = mybir.dt.bfloat16
    Exp = mybir.ActivationFunctionType.Exp
    MULT = mybir.AluOpType.mult
    ADD = mybir.AluOpType.add
    AX = mybir.AxisListType.X

    nc = bacc.Bacc("TRN2", target_bir_lowering=False, debug=False,
                   enable_asserts=False, num_devices=8)

    din = {}
    for name, n, dt in [
        ("xq4r", NQ4, f32r), ("xk4r", N4, f32r), ("xk4t", N4, bf16),
        ("xq2r", NQ2, f32r), ("xk2r", N2, f32r), ("xk2t", N2, bf16),
        ("xk1r", N1, f32r), ("xk1t", N1, bf16),
    ]:
        din[name] = nc.dram_tensor(name, [P, n], dt, kind="ExternalInput").ap()
    din["negb4"] = nc.dram_tensor("negb4", [P, 2], f32, kind="ExternalInput").ap()
    din["negb2"] = nc.dram_tensor("negb2", [P, 5], f32, kind="ExternalInput").ap()
    din["negb1"] = nc.dram_tensor("negb1", [P, 16], f32, kind="ExternalInput").ap()
    din["identf"] = nc.dram_tensor("identf", [P, P], f32, kind="ExternalInput").ap()
    out_d = nc.dram_tensor("out", [P, NQ1], f32, kind="ExternalOutput").ap()

    with tile.TileContext(nc) as tc:
        with (
            tc.tile_pool(name="sb_in", bufs=1) as sb_in,
            tc.tile_pool(name="sb_attn", bufs=2) as sb_attn,
            tc.tile_pool(name="sb_ac", bufs=6) as sb_ac,
            tc.tile_pool(name="sb_work", bufs=2) as sb_work,
            tc.tile_pool(name="sb_small", bufs=4) as sb_small,
            tc.tile_pool(name="sb_out", bufs=1) as sb_out,
            tc.tile_pool(name="sb_up", bufs=1) as sb_up,
            tc.tile_pool(name="ps_sc", bufs=SC_BUFS, space="PSUM") as ps_sc,
            tc.tile_pool(name="ps_out", bufs=2, space="PSUM") as ps_out,
        ):
            # ---- load inputs (small scales first so compute can start) ----
            t = {}
            order = ["identf", "negb4", "xq4r", "xk4r", "xk4t",
                     "negb2", "xq2r", "xk2r", "xk2t",
                     "negb1", "xk1r", "xk1t"]
            for name in order:
                ap = din[name]
                tl = sb_in.tile(list(ap.shape), ap.dtype, tag=name)
                nc.sync.dma_start(out=tl[:], in_=ap)
                t[name] = tl
            # warm the ACT exp table while input DMAs stream: walrus puts
            # the table load before the first ACTIVATE in program order
            warm = sb_small.tile([P, 1], f32, tag="warm", name="warm")
            nc.scalar.activation(warm[:, :], t["negb4"][:, 0:1], Exp)

            # two persistent PSUM banks: group g accumulates phase B in
            # psAB[g%2] while its 1/denom transposes stage in psAB[(g+1)%2]
            psAB = [ps_out.tile([P, 512], f32, tag="outps", name="psA"),
                    ps_out.tile([P, 512], f32, tag="outps", name="psB")]

            # warm the PE p-state during the input DMA window (only needs
            # identf, the first DMA): dummy matmuls so the real stream
            # starts closer to max clock
            for _ in range(WARMUP_MM):
                nc.tensor.matmul(psAB[0][:, 0:P], lhsT=t["identf"][:, :],
                                 rhs=t["identf"][:, :], start=True, stop=True)

            out_sb = sb_out.tile([P, NQ1], f32, tag="out_sb")
            out2_sb = sb_out.tile([P, NQ2], f32, tag="out2_sb")
            out4_sb = sb_out.tile([P, NQ4], f32, tag="out4_sb")

            def phase_a_units(xq_r, xk_r, negb, Nkv, g0, w, attnT):
                """Closures for group (g0,w) phase A: score chunks + exp.
                Denominators: ACT accum_out for the big chunks, DVE reduce
                for the 1024 tail. Returns (units, state)."""
                if Nkv > MCHUNK:
                    chunks = [(0, 1536), (1536, 1536), (3072, 1024)]
                    assert Nkv == 4096
                else:
                    chunks = [(0, Nkv)]
                nchunks = len(chunks)
                units = []
                recs = []
                state = {"recs": recs, "nchunks": nchunks, "w": w}

                def qtile_chunk(k, mc):
                    qt = g0 + k
                    off, csz = chunks[mc]
                    ps = ps_sc.tile([P, 1536], f32, tag="scores", name="ps")
                    for s0 in range(0, csz, 512):
                        sw = min(512, csz - s0)
                        nc.tensor.matmul(
                            ps[:, s0:s0 + sw],
                            lhsT=xq_r[:, qt * P:(qt + 1) * P],
                            rhs=xk_r[:, off + s0: off + s0 + sw],
                            start=True, stop=True)
                    ac = sb_ac.tile([P, 1536], bf16, tag="attnc", name="ac")
                    on_dve = (nchunks > 1 and mc == nchunks - 1)
                    nc.scalar.activation(
                        ac[:, :csz], ps[:, :csz], Exp,
                        bias=negb[:, qt:qt + 1],
                        accum_out=None if on_dve else recs[k][:, mc:mc + 1])
                    nc.sync.dma_start_transpose(
                        out=attnT[:, off // P: off // P + csz // P,
                                  k * P:(k + 1) * P],
                        in_=ac[:, :csz])
                    if on_dve:
                        nc.vector.reduce_sum(recs[k][:, mc:mc + 1],
                                             ac[:, :csz], axis=AX)

                def qtile_first(k):
                    if k == 0:
                        state["rec4"] = sb_small.tile([P, 4], f32, tag="rec4", name="rec4")
                    recs.append(sb_small.tile([P, 4], f32, tag="parts", name="parts"))
                    qtile_chunk(k, 0)

                def qtile_finish(k):
                    rec4 = state["rec4"]
                    if nchunks > 1:
                        denom = sb_small.tile([P, 1], f32, tag="denom")
                        nc.vector.reduce_sum(denom[:, :], recs[k][:, 0:nchunks], axis=AX)
                        nc.vector.reciprocal(rec4[:, k:k + 1], denom[:, :])
                    else:
                        nc.vector.reciprocal(rec4[:, k:k + 1], recs[k][:, 0:1])

                # chunk-0 of every q-tile FIRST: their DMA transposes gate
                # the next group's phase-B start, so fire them early
                chunk_units = []
                for k in range(w):
                    chunk_units.append(lambda k=k: qtile_first(k))
                for k in range(w):
                    for mc in range(1, nchunks):
                        chunk_units.append(lambda k=k, mc=mc: qtile_chunk(k, mc))
                # weave finishes in, 2 chunk-units after each q-tile's last chunk
                fins = {}
                for k in range(w):
                    pos = (w - 1 + (k + 1) * (nchunks - 1) + 2) if nchunks > 1 \
                        else (k + 2)
                    fins.setdefault(pos, []).append(lambda k=k: qtile_finish(k))
                for i, cu in enumerate(chunk_units):
                    units.append(cu)
                    for f in fins.pop(i, []):
                        units.append(f)
                for pos in sorted(fins):
                    units.extend(fins[pos])
                return units, state

            def phase_b_units(xk_t, out_dst, Nkv, g0, w, attnT, out_ps,
                              stage_ps, state):
                """Closures for group (g0,w) phase B: accumulating matmuls,
                with the 1/denom broadcast woven in (PE transposes stage in
                the other PSUM bank; partition broadcast on GpSimd), then
                the normalizing store."""
                n_mt = Nkv // P
                units = []
                bcast_ref = []

                def pair_step(mt0):
                    npair = min(2, n_mt - mt0)
                    for i in range(npair):
                        mt = mt0 + i
                        nc.tensor.matmul(out_ps[:, :w * P],
                                         lhsT=xk_t[:, mt * P:(mt + 1) * P],
                                         rhs=attnT[:, mt, :w * P],
                                         start=(mt == 0),
                                         stop=(mt == n_mt - 1))

                def bcast_unit():
                    rec4 = state["rec4"]
                    for k in range(w):
                        nc.tensor.transpose(stage_ps[0:1, k * P:(k + 1) * P],
                                            rec4[:, k:k + 1], t["identf"][:, :])
                    rec_row = sb_small.tile([1, 512], f32, tag="recrow")
                    nc.vector.tensor_copy(rec_row[0:1, :w * P],
                                          stage_ps[0:1, :w * P])
                    bcast = sb_work.tile([P, 512], f32, tag="bcast")
                    nc.gpsimd.partition_broadcast(bcast[:, :w * P],
                                                  rec_row[0:1, :w * P])
                    bcast_ref.append(bcast)

                for mt0 in range(0, n_mt, 2):
                    units.append(lambda mt0=mt0: pair_step(mt0))
                units.insert(min(BCAST_POS, len(units)), bcast_unit)

                def store_unit():
                    nc.vector.tensor_tensor(out_dst[:, g0 * P:(g0 + w) * P],
                                            out_ps[:, :w * P], bcast_ref[0][:, :w * P],
                                            MULT)
                    if out_dst is out_sb:
                        up = upsum_ref.get("ap")
                        if up is None:
                            s1_unfused.append((g0, w))
                        else:
                            nc.vector.tensor_tensor(
                                out_dst[:, g0 * P:(g0 + w) * P],
                                out_dst[:, g0 * P:(g0 + w) * P],
                                up[:, g0 * P:(g0 + w) * P], ADD)
                            if g0 + w == 12:  # groups 2 done -> flush cols
                                nc.sync.dma_start(out=out_d[:, 1024:1536],
                                                  in_=out_sb[:, 1024:1536])
                units.append(store_unit)
                return units

            pending_b = []
            upsum_ref = {}
            s1_unfused = []
            gctr = {"g": 0}

            def emit_interleaved(units_a, units_b):
                """Emit A units (current group) interleaved with B units
                (previous group) so the static engine streams alternate."""
                na, nb = len(units_a), len(units_b)
                bi = 0
                for ai, ua in enumerate(units_a):
                    ua()
                    want = ((ai + 1) * nb) // na
                    while bi < want:
                        units_b[bi]()
                        bi += 1
                while bi < nb:
                    units_b[bi]()
                    bi += 1

            def scale_attn(xq_r, xk_r, xk_t, negb, out_dst, Nkv, Nq, groups,
                           extra_after=None):
                for gi, (g0, w) in enumerate(groups):
                    g = gctr["g"]
                    gctr["g"] += 1
                    attnT = sb_attn.tile([P, Nkv // P, 512], bf16,
                                         tag="attnT", name="attnT")
                    out_ps, stage_ps = psAB[g % 2], psAB[(g + 1) % 2]
                    ua, state = phase_a_units(xq_r, xk_r, negb, Nkv, g0, w,
                                              attnT)
                    if len(ua) >= 6 or not pending_b:
                        emit_interleaved(ua, pending_b[:])
                        del pending_b[:]
                    else:
                        # tiny A stream (scale transition): emit it alone and
                        # keep B pending so the in-order tensor queue isn't
                        # blocked behind B's transpose waits
                        for u in ua:
                            u()
                    pending_b.extend(
                        phase_b_units(xk_t, out_dst, Nkv, g0, w, attnT,
                                      out_ps, stage_ps, state))
                    if extra_after and gi in extra_after:
                        pending_b.extend(extra_after[gi])

            # ---- upsample + cross-scale sum, as interleavable units ----
            ust = {}

            def up4_a():
                x4v = out4_sb.rearrange("p (h w) -> p h w", w=16)
                b4 = sb_up.tile([P, 16, 16], f32, tag="b4")     # 0.625 * in
                d4 = sb_up.tile([P, 16, 16], f32, tag="d4")     # 0.875 * in
                nc.vector.tensor_scalar_mul(b4[:], x4v[:, :, :], 0.625)
                nc.vector.tensor_scalar_mul(d4[:], x4v[:, :, :], 0.875)
                h4 = sb_up.tile([P, 8, 4, 16], f32, tag="h4")   # [j, phase, w]
                nc.vector.scalar_tensor_tensor(h4[:, :, 0, :], x4v[:, 0:8, :], 0.375,
                                               b4[:, 1:9, :], MULT, ADD)
                nc.vector.scalar_tensor_tensor(h4[:, :, 1, :], x4v[:, 0:8, :], 0.125,
                                               d4[:, 1:9, :], MULT, ADD)
                nc.vector.scalar_tensor_tensor(h4[:, :, 2, :], x4v[:, 2:10, :], 0.125,
                                               d4[:, 1:9, :], MULT, ADD)
                nc.vector.scalar_tensor_tensor(h4[:, :, 3, :], x4v[:, 2:10, :], 0.375,
                                               b4[:, 1:9, :], MULT, ADD)
                ust["h4f"] = h4.rearrange("p j q w -> p (j q) w")  # [32 rows, 16]

            def up4_b():
                h4f = ust["h4f"]
                b4w = sb_up.tile([P, 32, 16], f32, tag="b4w")
                d4w = sb_up.tile([P, 32, 16], f32, tag="d4w")
                nc.vector.tensor_scalar_mul(b4w[:], h4f[:, :, :], 0.625)
                nc.vector.tensor_scalar_mul(d4w[:], h4f[:, :, :], 0.875)
                up4 = sb_up.tile([P, 32, 16, 4], f32, tag="up4")  # [row, j, phase]
                nc.vector.scalar_tensor_tensor(up4[:, :, 1:16, 0], h4f[:, :, 0:15], 0.375,
                                               b4w[:, :, 1:16], MULT, ADD)
                nc.vector.scalar_tensor_tensor(up4[:, :, 1:16, 1], h4f[:, :, 0:15], 0.125,
                                               d4w[:, :, 1:16], MULT, ADD)
                nc.vector.scalar_tensor_tensor(up4[:, :, 0:15, 2], h4f[:, :, 1:16], 0.125,
                                               d4w[:, :, 0:15], MULT, ADD)
                nc.vector.scalar_tensor_tensor(up4[:, :, 0:15, 3], h4f[:, :, 1:16], 0.375,
                                               b4w[:, :, 0:15], MULT, ADD)
                nc.vector.tensor_copy(up4[:, :, 0:1, 0], h4f[:, :, 0:1])
                nc.vector.tensor_copy(up4[:, :, 0:1, 1], h4f[:, :, 0:1])
                nc.vector.tensor_copy(up4[:, :, 15:16, 2], h4f[:, :, 15:16])
                nc.vector.tensor_copy(up4[:, :, 15:16, 3], h4f[:, :, 15:16])
                ust["up4"] = up4

            def up2_a():
                x2v = out2_sb.rearrange("p (h w) -> p h w", w=32)
                b2 = sb_up.tile([P, 20, 32], f32, tag="b2")     # 0.75 * in
                nc.vector.tensor_scalar_mul(b2[:], x2v[:, :, :], 0.75)
                h2 = sb_up.tile([P, 16, 2, 32], f32, tag="h2")
                nc.vector.scalar_tensor_tensor(h2[:, :, 0, :], x2v[:, 0:16, :], 0.25,
                                               b2[:, 1:17, :], MULT, ADD)
                nc.vector.scalar_tensor_tensor(h2[:, :, 1, :], x2v[:, 2:18, :], 0.25,
                                               b2[:, 1:17, :], MULT, ADD)
                ust["h2f"] = h2.rearrange("p j q w -> p (j q) w")  # [32 rows, 32]

            def up2_b():
                h2f = ust["h2f"]
                b2w = sb_up.tile([P, 32, 32], f32, tag="b2w")
                nc.vector.tensor_scalar_mul(b2w[:], h2f[:, :, :], 0.75)
                up2 = sb_up.tile([P, 32, 32, 2], f32, tag="up2")
                nc.vector.scalar_tensor_tensor(up2[:, :, 1:32, 0], h2f[:, :, 0:31], 0.25,
                                               b2w[:, :, 1:32], MULT, ADD)
                nc.vector.scalar_tensor_tensor(up2[:, :, 0:31, 1], h2f[:, :, 1:32], 0.25,
                                               b2w[:, :, 0:31], MULT, ADD)
                nc.vector.tensor_copy(up2[:, :, 0:1, 0], h2f[:, :, 0:1])
                nc.vector.tensor_copy(up2[:, :, 31:32, 1], h2f[:, :, 31:32])
                # upsum = up4 + up2, flattened to match out_sb columns
                up4f = ust["up4"].rearrange("p h j q -> p (h j q)")
                up2f = up2.rearrange("p h j q -> p (h j q)")
                nc.vector.tensor_tensor(up4f[:, :], up4f[:, :], up2f[:, :], ADD)
                upsum_ref["ap"] = up4f

            def up_flush():
                up4f = upsum_ref["ap"]
                hi = 0
                for g0w, ww in s1_unfused:
                    nc.vector.tensor_tensor(
                        out_sb[:, g0w * P:(g0w + ww) * P],
                        out_sb[:, g0w * P:(g0w + ww) * P],
                        up4f[:, g0w * P:(g0w + ww) * P], ADD)
                    hi = max(hi, (g0w + ww) * P)
                del s1_unfused[:]
                if hi:
                    nc.sync.dma_start(out=out_d[:, 0:hi], in_=out_sb[:, 0:hi])

            scale_attn(t["xq4r"], t["xk4r"], t["xk4t"], t["negb4"], out4_sb,
                       N4, NQ4, [(0, 2)])
            scale_attn(t["xq2r"], t["xk2r"], t["xk2t"], t["negb2"], out2_sb,
                       N2, NQ2, [(0, 4), (4, 1)])
            scale_attn(t["xk1r"], t["xk1r"], t["xk1t"], t["negb1"], out_sb,
                       N1, NQ1, S1_GROUPS,
                       extra_after={0: [up4_a, up4_b],
                                    1: [up2_a, up2_b, up_flush]})
            for ub in pending_b:
                ub()
            del pending_b[:]

            # ---- store the remaining columns ----
            nc.sync.dma_start(out=out_d[:, 1536:2048], in_=out_sb[:, 1536:2048])

    nc.compile()
    return nc


_NC = None


def _get_nc():
    global _NC
    if _NC is None:
        _NC = _build_module()
    return _NC


def _pool(x64, s):
    Bs, Cs, Hs, Ws = x64.shape
    return x64.reshape(Bs, Cs, Hs // s, s, Ws // s, s).mean(axis=(3, 5))


def host_prep(x):
    """Build the 8 per-core input maps from the full x [4,128,64,64] f32."""
    x64 = np.asarray(x, dtype=np.float64)
    p1 = np.asarray(x, dtype=np.float32).reshape(B, C, N1)
    p2 = _pool(x64, 2).astype(np.float32).reshape(B, C, N2)
    p4 = _pool(x64, 4).astype(np.float32).reshape(B, C, N4)

    ident_f = np.eye(P, dtype=np.float32)

    def kt(pool_flat):
        # [C, N] -> bf16 [P, (mt, c)] with kt[p, mt*128+c] = pool[c, mt*128+p]
        n = pool_flat.shape[1]
        return (pool_flat.T.reshape(n // P, P, C).transpose(1, 0, 2)
                .reshape(P, n).astype(_BF16))

    def negb_of(pool_flat, cols):
        norms = np.sqrt((pool_flat.astype(np.float64) ** 2).sum(0))
        Xm = norms.max()
        # Cauchy-Schwarz bound |x_q|*Xm can overshoot the true row max by
        # >88, underflowing every exp() in the row (denom=0 -> NaN). The
        # row max is >= the diagonal |x_q|^2, so clamp the bias there +30:
        # keeps exp(rowmax-bias) >= e^-30 while exp(score-bias) stays
        # bounded by e^(rowmax - |x_q|^2 - 30), small for this data.
        nb = -np.minimum(norms[cols] * Xm, norms[cols] ** 2 + 30.0)
        ntile = len(cols) // P
        return nb.reshape(ntile, P).T.astype(np.float32).copy()

    in_maps = []
    for b in range(B):
        for h in (0, 1):
            # query columns per scale (with clamped overlap rows)
            r2 = np.clip(h * 16 - 1 + np.arange(20), 0, 31)
            q2 = (r2[:, None] * 32 + np.arange(32)[None, :]).ravel()
            r4 = np.clip(h * 8 - 1 + np.arange(16), 0, 15)
            q4 = (r4[:, None] * 16 + np.arange(16)[None, :]).ravel()
            # scale 1: rotate keys so this core's queries are cols [0,NQ1)
            p1r = np.roll(p1[b], -h * NQ1, axis=1) if h else p1[b]
            m = {
                "xk1r": p1r.copy(),
                "xk1t": kt(p1r), "negb1": negb_of(p1r, np.arange(NQ1)),
                "xq2r": p2[b][:, q2].copy(), "xk2r": p2[b].copy(),
                "xk2t": kt(p2[b]), "negb2": negb_of(p2[b], q2),
                "xq4r": p4[b][:, q4].copy(), "xk4r": p4[b].copy(),
                "xk4t": kt(p4[b]), "negb4": negb_of(p4[b], q4),
                "identf": ident_f,
            }
            in_maps.append(m)
    return in_maps


def assemble(results):
    """results: list of 8 dicts with 'out' [128, 2048] -> full [4,128,64,64]."""
    out = np.empty((B, C, H, W), np.float32)
    for b in range(B):
        for h in (0, 1):
            core = results[2 * b + h]["out"]
            out[b, :, h * 32:(h + 1) * 32, :] = core.reshape(C, 32, W)
    return out


def kernel(x):
    from concourse.bass_utils import run_bass_kernel_spmd

    nc = _get_nc()
    in_maps = host_prep(np.asarray(x, dtype=np.float32))
    res = run_bass_kernel_spmd(nc, in_maps, core_ids=list(range(8)))
    return assemble(res.results)


# revision 11
# speedup vs baseline: 1.0120x; 1.0120x over previous
"""Multi-scale self-attention (nn_AttentionModule) as a Bass/Tile kernel
on 8 TRN2 NeuronCores.

Problem: for scales (4,2,1): avg-pool x [4,128,64,64] -> [B,C,Hs,Ws],
N=Hs*Ws self-attention with q=k=v=x (C=128 contraction), bilinear
upsample back to 64x64 (half-pixel, edge-clamped), sum over scales.

Sharding: 2 cores per batch element; each core computes half the
queries at every scale (with one overlap row at the coarse scales so
the bilinear upsample is core-local) and produces rows [h*32,(h+1)*32)
of its batch's output. All cores run the identical program; only the
input data differs. For scale 1 the host ROTATES the key order per
core so the query block is always columns [0,2048) of xk1r — no
separate xq1r tensor (attention is permutation-invariant in m).

Per-core algorithm (per scale):
 - scores tile [q=128, m<=1536] = xq^T @ xk  (fp32r matmuls, K=C=128)
 - softmax without a max pass: bias b_q = min(|x_q| * max_m |x_m|,
   |x_q|^2 + 30) >= row max (Cauchy-Schwarz / diagonal), applied as the
   per-partition ACT bias; exp emits unnormalized attn in bf16 with the
   row-sum denominator via accum_out (the small tail chunk sums on DVE
   to spare scalar-engine accumulator reads).
 - attn chunks are DMA-transposed to [m-tile, q] layout; phase B
   accumulates out_unnorm[c,q] = sum_m xkT[m,c]*attnT[m,q].
 - two persistent PSUM banks alternate per group: one accumulates phase
   B while the other stages the 1/denom PE transposes; the partition
   broadcast of 1/denom runs on the (idle) GpSimd engine, so the
   normalizing store costs O(C*N) and never blocks the tensor queue.
 - bilinear upsample + cross-scale sum with strided DVE ops, emitted
   interleaved with scale-1 compute; output DMA split per column range.
"""

import numpy as np
import ml_dtypes

MCHUNK = 1024
S1_GROUPS = [(0, 4), (4, 4), (8, 4), (12, 4)]
SC_BUFS = 2
WARMUP_MM = 10
BCAST_POS = 4          # pair-steps before the bcast unit in phase B
DEFER_B = 5            # B units carried into the following window

P = 128
B, C, H, W = 4, 128, 64, 64
N1, N2, N4 = 4096, 1024, 256
NQ1 = 2048          # 16 q-tiles (half the image)
NQ2 = 640           # 5 q-tiles = 20 pooled rows (18 needed + 2 pad)
NQ4 = 256           # 2 q-tiles = 16 pooled rows (10 needed + 6 pad)

_BF16 = ml_dtypes.bfloat16


def _build_module():
    import concourse.bacc as bacc
    import concourse.mybir as mybir
    import concourse.tile as tile

    f32 = mybir.dt.float32
    f32r = mybir.dt.float32r
    bf16 = mybir.dt.bfloat16
    Exp = mybir.ActivationFunctionType.Exp
    MULT = mybir.AluOpType.mult
    ADD = mybir.AluOpType.add
    AX = mybir.AxisListType.X

    nc = bacc.Bacc("TRN2", target_bir_lowering=False, debug=False,
                   enable_asserts=False, num_devices=8)

    din = {}
    for name, n, dt in [
        ("xq4r", NQ4, f32r), ("xk4r", N4, f32r), ("xk4t", N4, bf16),
        ("xq2r", NQ2, f32r), ("xk2r", N2, f32r), ("xk2t", N2, bf16),
        ("xk1r", N1, f32r), ("xk1t", N1, bf16),
    ]:
        din[name] = nc.dram_tensor(name, [P, n], dt, kind="ExternalInput").ap()
    din["negb4"] = nc.dram_tensor("negb4", [P, 2], f32, kind="ExternalInput").ap()
    din["negb2"] = nc.dram_tensor("negb2", [P, 5], f32, kind="ExternalInput").ap()
    din["negb1"] = nc.dram_tensor("negb1", [P, 16], f32, kind="ExternalInput").ap()
    din["identf"] = nc.dram_tensor("identf", [P, P], f32, kind="ExternalInput").ap()
    out_d = nc.dram_tensor("out", [P, NQ1], f32, kind="ExternalOutput").ap()

    with tile.TileContext(nc) as tc:
        with (
            tc.tile_pool(name="sb_in", bufs=1) as sb_in,
            tc.tile_pool(name="sb_attn", bufs=2) as sb_attn,
            tc.tile_pool(name="sb_ac", bufs=6) as sb_ac,
            tc.tile_pool(name="sb_work", bufs=2) as sb_work,
            tc.tile_pool(name="sb_small", bufs=4) as sb_small,
            tc.tile_pool(name="sb_out", bufs=1) as sb_out,
            tc.tile_pool(name="sb_up", bufs=1) as sb_up,
            tc.tile_pool(name="ps_sc", bufs=SC_BUFS, space="PSUM") as ps_sc,
            tc.tile_pool(name="ps_out", bufs=2, space="PSUM") as ps_out,
        ):
            # ---- load inputs (small scales first so compute can start) ----
            t = {}
            order = ["identf", "negb4", "xq4r", "xk4r", "xk4t",
                     "negb2", "xq2r", "xk2r", "xk2t",
                     "negb1", "xk1r", "xk1t"]
            for name in order:
                ap = din[name]
                tl = sb_in.tile(list(ap.shape), ap.dtype, tag=name)
                nc.sync.dma_start(out=tl[:], in_=ap)
                t[name] = tl
            # warm the ACT exp table while input DMAs stream: walrus puts
            # the table load before the first ACTIVATE in program order
            warm = sb_small.tile([P, 1], f32, tag="warm", name="warm")
            nc.scalar.activation(warm[:, :], t["negb4"][:, 0:1], Exp)

            # two persistent PSUM banks: group g accumulates phase B in
            # psAB[g%2] while its 1/denom transposes stage in psAB[(g+1)%2]
            psAB = [ps_out.tile([P, 512], f32, tag="outps", name="psA"),
                    ps_out.tile([P, 512], f32, tag="outps", name="psB")]

            # warm the PE p-state during the input DMA window (only needs
            # identf, the first DMA): dummy matmuls so the real stream
            # starts closer to max clock
            for _ in range(WARMUP_MM):
                nc.tensor.matmul(psAB[0][:, 0:P], lhsT=t["identf"][:, :],
                                 rhs=t["identf"][:, :], start=True, stop=True)

            out_sb = sb_out.tile([P, NQ1], f32, tag="out_sb")
            out2_sb = sb_out.tile([P, NQ2], f32, tag="out2_sb")
            out4_sb = sb_out.tile([P, NQ4], f32, tag="out4_sb")

            def phase_a_units(xq_r, xk_r, negb, Nkv, g0, w, attnT):
                """Closures for group (g0,w) phase A: score chunks + exp.
                Denominators: ACT accum_out for the big chunks, DVE reduce
                for the 1024 tail. Returns (units, state)."""
                if Nkv > MCHUNK:
                    chunks = [(0, 1024), (1024, 1536), (2560, 1536)]
                    assert Nkv == 4096
                else:
                    chunks = [(0, Nkv)]
                nchunks = len(chunks)
                units = []
                recs = []
                state = {"recs": recs, "nchunks": nchunks, "w": w}

                def qtile_chunk(k, mc):
                    qt = g0 + k
                    off, csz = chunks[mc]
                    ps = ps_sc.tile([P, 1536], f32, tag="scores", name="ps")
                    for s0 in range(0, csz, 512):
                        sw = min(512, csz - s0)
                        nc.tensor.matmul(
                            ps[:, s0:s0 + sw],
                            lhsT=xq_r[:, qt * P:(qt + 1) * P],
                            rhs=xk_r[:, off + s0: off + s0 + sw],
                            start=True, stop=True)
                    ac = sb_ac.tile([P, 1536], bf16, tag="attnc", name="ac")
                    on_dve = (nchunks > 1 and mc == nchunks - 1)
                    nc.scalar.activation(
                        ac[:, :csz], ps[:, :csz], Exp,
                        bias=negb[:, qt:qt + 1],
                        accum_out=None if on_dve else recs[k][:, mc:mc + 1])
                    nc.sync.dma_start_transpose(
                        out=attnT[:, off // P: off // P + csz // P,
                                  k * P:(k + 1) * P],
                        in_=ac[:, :csz])
                    if on_dve:
                        nc.vector.reduce_sum(recs[k][:, mc:mc + 1],
                                             ac[:, :csz], axis=AX)

                def qtile_first(k):
                    if k == 0:
                        state["rec4"] = sb_small.tile([P, 4], f32, tag="rec4", name="rec4")
                    recs.append(sb_small.tile([P, 4], f32, tag="parts", name="parts"))
                    qtile_chunk(k, 0)

                def qtile_finish(k):
                    rec4 = state["rec4"]
                    if nchunks > 1:
                        denom = sb_small.tile([P, 1], f32, tag="denom")
                        nc.vector.reduce_sum(denom[:, :], recs[k][:, 0:nchunks], axis=AX)
                        nc.vector.reciprocal(rec4[:, k:k + 1], denom[:, :])
                    else:
                        nc.vector.reciprocal(rec4[:, k:k + 1], recs[k][:, 0:1])

                chunk_units = []
                for k in range(w):
                    chunk_units.append(lambda k=k: qtile_first(k))
                    for mc in range(1, nchunks):
                        chunk_units.append(lambda k=k, mc=mc: qtile_chunk(k, mc))
                # weave finishes in, 2 chunk-units after each q-tile's last chunk
                fins = {}
                for k in range(w):
                    pos = (k + 1) * nchunks - 1 + 2
                    fins.setdefault(pos, []).append(lambda k=k: qtile_finish(k))
                for i, cu in enumerate(chunk_units):
                    units.append(cu)
                    for f in fins.pop(i, []):
                        units.append(f)
                for pos in sorted(fins):
                    units.extend(fins[pos])
                return units, state

            def phase_b_units(xk_t, out_dst, Nkv, g0, w, attnT, out_ps,
                              stage_ps, state):
                """Closures for group (g0,w) phase B: accumulating matmuls,
                with the 1/denom broadcast woven in (PE transposes stage in
                the other PSUM bank; partition broadcast on GpSimd), then
                the normalizing store."""
                n_mt = Nkv // P
                units = []
                bcast_ref = []

                def pair_step(mt0):
                    npair = min(2, n_mt - mt0)
                    for i in range(npair):
                        mt = mt0 + i
                        nc.tensor.matmul(out_ps[:, :w * P],
                                         lhsT=xk_t[:, mt * P:(mt + 1) * P],
                                         rhs=attnT[:, mt, :w * P],
                                         start=(mt == 0),
                                         stop=(mt == n_mt - 1))

                def bcast_unit():
                    rec4 = state["rec4"]
                    for k in range(w):
                        nc.tensor.transpose(stage_ps[0:1, k * P:(k + 1) * P],
                                            rec4[:, k:k + 1], t["identf"][:, :])
                    rec_row = sb_small.tile([1, 512], f32, tag="recrow")
                    nc.vector.tensor_copy(rec_row[0:1, :w * P],
                                          stage_ps[0:1, :w * P])
                    bcast = sb_work.tile([P, 512], f32, tag="bcast")
                    nc.gpsimd.partition_broadcast(bcast[:, :w * P],
                                                  rec_row[0:1, :w * P])
                    bcast_ref.append(bcast)

                for mt0 in range(0, n_mt, 2):
                    units.append(lambda mt0=mt0: pair_step(mt0))
                units.insert(min(BCAST_POS, len(units)), bcast_unit)

                def store_unit():
                    nc.vector.tensor_tensor(out_dst[:, g0 * P:(g0 + w) * P],
                                            out_ps[:, :w * P], bcast_ref[0][:, :w * P],
                                            MULT)
                    if out_dst is out_sb:
                        up = upsum_ref.get("ap")
                        if up is None:
                            s1_unfused.append((g0, w))
                        else:
                            nc.vector.tensor_tensor(
                                out_dst[:, g0 * P:(g0 + w) * P],
                                out_dst[:, g0 * P:(g0 + w) * P],
                                up[:, g0 * P:(g0 + w) * P], ADD)
                            if g0 + w == 12:  # groups 2 done -> flush cols
                                nc.sync.dma_start(out=out_d[:, 1024:1536],
                                                  in_=out_sb[:, 1024:1536])
                units.append(store_unit)
                return units

            pending_b = []
            upsum_ref = {}
            s1_unfused = []
            gctr = {"g": 0}

            def emit_interleaved(units_a, units_b, shift=3):
                """Emit A units (current group) interleaved with B units
                (previous group) so the static engine streams alternate.
                B units start `shift` A-units in (their transposes need the
                previous group's last exps to have drained)."""
                na, nb = len(units_a), len(units_b)
                sh = min(shift, max(na - nb, 0)) if na > 4 else 0
                bi = 0
                for ai, ua in enumerate(units_a):
                    ua()
                    want = ((ai + 1 - sh) * nb) // (na - sh) if ai + 1 > sh else 0
                    while bi < want:
                        units_b[bi]()
                        bi += 1
                while bi < nb:
                    units_b[bi]()
                    bi += 1

            def scale_attn(xq_r, xk_r, xk_t, negb, out_dst, Nkv, Nq, groups,
                           extra_after=None):
                for gi, (g0, w) in enumerate(groups):
                    g = gctr["g"]
                    gctr["g"] += 1
                    attnT = sb_attn.tile([P, Nkv // P, 512], bf16,
                                         tag="attnT", name="attnT")
                    out_ps, stage_ps = psAB[g % 2], psAB[(g + 1) % 2]
                    ua, state = phase_a_units(xq_r, xk_r, negb, Nkv, g0, w,
                                              attnT)
                    if len(ua) >= 6 or not pending_b:
                        # defer a tail of B units into the NEXT window: their
                        # transposes are the freshest and the deferred units
                        # give the final flush transpose-ready work
                        ndefer = DEFER_B if len(pending_b) > DEFER_B + 2 else 0
                        emit_interleaved(ua, pending_b[:len(pending_b) - ndefer])
                        del pending_b[:len(pending_b) - ndefer]
                    else:
                        # tiny A stream (scale transition): emit it alone and
                        # keep B pending so the in-order tensor queue isn't
                        # blocked behind B's transpose waits
                        for u in ua:
                            u()
                    pending_b.extend(
                        phase_b_units(xk_t, out_dst, Nkv, g0, w, attnT,
                                      out_ps, stage_ps, state))
                    if extra_after and gi in extra_after:
                        pending_b.extend(extra_after[gi])

            # ---- upsample + cross-scale sum, as interleavable units ----
            ust = {}

            def up4_a():
                x4v = out4_sb.rearrange("p (h w) -> p h w", w=16)
                b4 = sb_up.tile([P, 16, 16], f32, tag="b4")     # 0.625 * in
                d4 = sb_up.tile([P, 16, 16], f32, tag="d4")     # 0.875 * in
                nc.vector.tensor_scalar_mul(b4[:], x4v[:, :, :], 0.625)
                nc.vector.tensor_scalar_mul(d4[:], x4v[:, :, :], 0.875)
                h4 = sb_up.tile([P, 8, 4, 16], f32, tag="h4")   # [j, phase, w]
                nc.vector.scalar_tensor_tensor(h4[:, :, 0, :], x4v[:, 0:8, :], 0.375,
                                               b4[:, 1:9, :], MULT, ADD)
                nc.vector.scalar_tensor_tensor(h4[:, :, 1, :], x4v[:, 0:8, :], 0.125,
                                               d4[:, 1:9, :], MULT, ADD)
                nc.vector.scalar_tensor_tensor(h4[:, :, 2, :], x4v[:, 2:10, :], 0.125,
                                               d4[:, 1:9, :], MULT, ADD)
                nc.vector.scalar_tensor_tensor(h4[:, :, 3, :], x4v[:, 2:10, :], 0.375,
                                               b4[:, 1:9, :], MULT, ADD)
                ust["h4f"] = h4.rearrange("p j q w -> p (j q) w")  # [32 rows, 16]

            def up4_b():
                h4f = ust["h4f"]
                b4w = sb_up.tile([P, 32, 16], f32, tag="b4w")
                d4w = sb_up.tile([P, 32, 16], f32, tag="d4w")
                nc.vector.tensor_scalar_mul(b4w[:], h4f[:, :, :], 0.625)
                nc.vector.tensor_scalar_mul(d4w[:], h4f[:, :, :], 0.875)
                up4 = sb_up.tile([P, 32, 16, 4], f32, tag="up4")  # [row, j, phase]
                nc.vector.scalar_tensor_tensor(up4[:, :, 1:16, 0], h4f[:, :, 0:15], 0.375,
                                               b4w[:, :, 1:16], MULT, ADD)
                nc.vector.scalar_tensor_tensor(up4[:, :, 1:16, 1], h4f[:, :, 0:15], 0.125,
                                               d4w[:, :, 1:16], MULT, ADD)
                nc.vector.scalar_tensor_tensor(up4[:, :, 0:15, 2], h4f[:, :, 1:16], 0.125,
                                               d4w[:, :, 0:15], MULT, ADD)
                nc.vector.scalar_tensor_tensor(up4[:, :, 0:15, 3], h4f[:, :, 1:16], 0.375,
                                               b4w[:, :, 0:15], MULT, ADD)
                nc.vector.tensor_copy(up4[:, :, 0:1, 0], h4f[:, :, 0:1])
                nc.vector.tensor_copy(up4[:, :, 0:1, 1], h4f[:, :, 0:1])
                nc.vector.tensor_copy(up4[:, :, 15:16, 2], h4f[:, :, 15:16])
                nc.vector.tensor_copy(up4[:, :, 15:16, 3], h4f[:, :, 15:16])
                ust["up4"] = up4

            def up2_a():
                x2v = out2_sb.rearrange("p (h w) -> p h w", w=32)
                b2 = sb_up.tile([P, 20, 32], f32, tag="b2")     # 0.75 * in
                nc.vector.tensor_scalar_mul(b2[:], x2v[:, :, :], 0.75)
                h2 = sb_up.tile([P, 16, 2, 32], f32, tag="h2")
                nc.vector.scalar_tensor_tensor(h2[:, :, 0, :], x2v[:, 0:16, :], 0.25,
                                               b2[:, 1:17, :], MULT, ADD)
                nc.vector.scalar_tensor_tensor(h2[:, :, 1, :], x2v[:, 2:18, :], 0.25,
                                               b2[:, 1:17, :], MULT, ADD)
                ust["h2f"] = h2.rearrange("p j q w -> p (j q) w")  # [32 rows, 32]

            def up2_b():
                h2f = ust["h2f"]
                b2w = sb_up.tile([P, 32, 32], f32, tag="b2w")
                nc.vector.tensor_scalar_mul(b2w[:], h2f[:, :, :], 0.75)
                up2 = sb_up.tile([P, 32, 32, 2], f32, tag="up2")
                nc.vector.scalar_tensor_tensor(up2[:, :, 1:32, 0], h2f[:, :, 0:31], 0.25,
                                               b2w[:, :, 1:32], MULT, ADD)
                nc.vector.scalar_tensor_tensor(up2[:, :, 0:31, 1], h2f[:, :, 1:32], 0.25,
                                               b2w[:, :, 0:31], MULT, ADD)
                nc.vector.tensor_copy(up2[:, :, 0:1, 0], h2f[:, :, 0:1])
                nc.vector.tensor_copy(up2[:, :, 31:32, 1], h2f[:, :, 31:32])
                # upsum = up4 + up2, flattened to match out_sb columns
                up4f = ust["up4"].rearrange("p h j q -> p (h j q)")
                up2f = up2.rearrange("p h j q -> p (h j q)")
                nc.vector.tensor_tensor(up4f[:, :], up4f[:, :], up2f[:, :], ADD)
                upsum_ref["ap"] = up4f

            def up_flush():
                up4f = upsum_ref["ap"]
                hi = 0
                for g0w, ww in s1_unfused:
                    nc.vector.tensor_tensor(
                        out_sb[:, g0w * P:(g0w + ww) * P],
                        out_sb[:, g0w * P:(g0w + ww) * P],
                        up4f[:, g0w * P:(g0w + ww) * P], ADD)
                    hi = max(hi, (g0w + ww) * P)
                del s1_unfused[:]
                if hi:
                    nc.sync.dma_start(out=out_d[:, 0:hi], in_=out_sb[:, 0:hi])

            scale_attn(t["xq4r"], t["xk4r"], t["xk4t"], t["negb4"], out4_sb,
                       N4, NQ4, [(0, 2)])
            scale_attn(t["xq2r"], t["xk2r"], t["xk2t"], t["negb2"], out2_sb,
                       N2, NQ2, [(0, 4), (4, 1)])
            scale_attn(t["xk1r"], t["xk1r"], t["xk1t"], t["negb1"], out_sb,
                       N1, NQ1, S1_GROUPS,
                       extra_after={0: [up4_a, up4_b],
                                    1: [up2_a, up2_b, up_flush]})
            for ub in pending_b:
                ub()
            del pending_b[:]

            # ---- store the remaining columns ----
            nc.sync.dma_start(out=out_d[:, 1536:2048], in_=out_sb[:, 1536:2048])

    nc.compile()
    return nc


_NC = None


def _get_nc():
    global _NC
    if _NC is None:
        _NC = _build_module()
    return _NC


def _pool(x64, s):
    Bs, Cs, Hs, Ws = x64.shape
    return x64.reshape(Bs, Cs, Hs // s, s, Ws // s, s).mean(axis=(3, 5))


def host_prep(x):
    """Build the 8 per-core input maps from the full x [4,128,64,64] f32."""
    x64 = np.asarray(x, dtype=np.float64)
    p1 = np.asarray(x, dtype=np.float32).reshape(B, C, N1)
    p2 = _pool(x64, 2).astype(np.float32).reshape(B, C, N2)
    p4 = _pool(x64, 4).astype(np.float32).reshape(B, C, N4)

    ident_f = np.eye(P, dtype=np.float32)

    def kt(pool_flat):
        # [C, N] -> bf16 [P, (mt, c)] with kt[p, mt*128+c] = pool[c, mt*128+p]
        n = pool_flat.shape[1]
        return (pool_flat.T.reshape(n // P, P, C).transpose(1, 0, 2)
                .reshape(P, n).astype(_BF16))

    def negb_of(pool_flat, cols):
        norms = np.sqrt((pool_flat.astype(np.float64) ** 2).sum(0))
        Xm = norms.max()
        # Cauchy-Schwarz bound |x_q|*Xm can overshoot the true row max by
        # >88, underflowing every exp() in the row (denom=0 -> NaN). The
        # row max is >= the diagonal |x_q|^2, so clamp the bias there +30:
        # keeps exp(rowmax-bias) >= e^-30 while exp(score-bias) stays
        # bounded by e^(rowmax - |x_q|^2 - 30), small for this data.
        nb = -np.minimum(norms[cols] * Xm, norms[cols] ** 2 + 30.0)
        ntile = len(cols) // P
        return nb.reshape(ntile, P).T.astype(np.float32).copy()

    in_maps = []
    for b in range(B):
        for h in (0, 1):
            # query columns per scale (with clamped overlap rows)
            r2 = np.clip(h * 16 - 1 + np.arange(20), 0, 31)
            q2 = (r2[:, None] * 32 + np.arange(32)[None, :]).ravel()
            r4 = np.clip(h * 8 - 1 + np.arange(16), 0, 15)
            q4 = (r4[:, None] * 16 + np.arange(16)[None, :]).ravel()
            # scale 1: rotate keys so this core's queries are cols [0,NQ1)
            p1r = np.roll(p1[b], -h * NQ1, axis=1) if h else p1[b]
            m = {
                "xk1r": p1r.copy(),
                "xk1t": kt(p1r), "negb1": negb_of(p1r, np.arange(NQ1)),
                "xq2r": p2[b][:, q2].copy(), "xk2r": p2[b].copy(),
                "xk2t": kt(p2[b]), "negb2": negb_of(p2[b], q2),
                "xq4r": p4[b][:, q4].copy(), "xk4r": p4[b].copy(),
                "xk4t": kt(p4[b]), "negb4": negb_of(p4[b], q4),
                "identf": ident_f,
            }
            in_maps.append(m)
    return in_maps


def assemble(results):
    """results: list of 8 dicts with 'out' [128, 2048] -> full [4,128,64,64]."""
    out = np.empty((B, C, H, W), np.float32)
    for b in range(B):
        for h in (0, 1):
            core = results[2 * b + h]["out"]
            out[b, :, h * 32:(h + 1) * 32, :] = core.reshape(C, 32, W)
    return out


def kernel(x):
    from concourse.bass_utils import run_bass_kernel_spmd

    nc = _get_nc()
    in_maps = host_prep(np.asarray(x, dtype=np.float32))
    res = run_bass_kernel_spmd(nc, in_maps, core_ids=list(range(8)))
    return assemble(res.results)
